# revision 79
# baseline (speedup 1.0000x reference)
"""MHSA over 32 independent 512-token segments, segment-parallel across 8
NeuronCores (4 segments / 2048 tokens per core, zero cross-core traffic).

QKV and output projection run as fp8e4m3 DoubleRow matmuls (0.5 cyc/row in
the PE) with hi+lo error compensation: every operand X is split host-side
into X = Xh + Xl (two fp8 planes, planar layout so the ldweights k-tile
stride meets the dual-fp8 ISA rule step%16==0), and X@W is computed as
Xh@Wh + (Xl@Wh + Xh@Wl), dropping only the lo*lo term.  Operands are
pre-scaled by powers of two (x*2^4, w*2^9) to center them in e4m3 range;
the scales are folded into the exp() argument, the A@V ones-column, and a
final tensor_scalar descale -- net precision is slightly BETTER than bf16.
A K=1024 output tile costs 12 DR insts x 256 cyc = 3072 cyc vs bf16's
8 x 512 = 4096.  x additionally arrives pre-transposed from the host
(x^T hi/lo planes), removing all on-device x transposes.

S = QK^T and A@V stay bf16 (contraction is 64/65-wide there; DoubleRow
with compensation degenerates to bf16 cost).  Per segment s, per head h:
  Q^T,K^T   DR fp8: lhsT=Wqk chunk, rhs=xT8          16x [128, 512]
  V         DR fp8, natural [tok, 1024] + s-col per head (A@V rowsum)
  S^T       = K^T' Q^T per head, 4x [64c -> 128k, 512q] into PSUM b0/b1
  A^T       = exp(S^T * 2^-26 / 8) bf16
  O'        = A^T.T @ [V_h|s]  natural [128q, 65] per qt -> bank 2+qt
  yt        = O'[:, 0:64] * recip(O'[:, 64]) per-partition scale (DVE)
  ytT8      = XBAR-transpose yt -> DVE hi/lo fp8 planes
  out       = ytT8 @ Wp8 (DR fp8) * 2^-13

PSUM banks: 0-1 S^T (ping-pong around exp), 2-5 A@V per qt, 6-7 QKV/proj
accumulation tiles.  3-stage software pipeline in PE emission order as in
the bf16 baseline (floor-paced worklist between attention heads).
"""

import numpy as np

import concourse.bass as bass
import concourse.mybir as mybir
import concourse.tile as tile
from concourse.bass_utils import run_bass_kernel_spmd

F32 = mybir.dt.float32
BF16 = mybir.dt.bfloat16
F8 = mybir.dt.float8e4
EXP = mybir.ActivationFunctionType.Exp
COPY = mybir.ActivationFunctionType.Copy
DR = mybir.MatmulPerfMode.DoubleRow

PHASE_MARKS = []


def _mark(nc, label):
    insts = list(nc.all_instructions())
    last = insts[-1].name if insts else "I-0"
    PHASE_MARKS.append((label, int(last.split("-")[1])))


T, C, H, HD = 16384, 1024, 16, 64
NCORES = 8
TOK = T // NCORES          # 2048 tokens per core
SEG = 512                  # tokens per segment
NSEG = TOK // SEG          # 4 segments per core
LGX, LGW = 4, 9            # x scaled by 2^4, weights by 2^9
LGQ = LGX + LGW            # Q/K/V scale 2^13
CY = 4                     # y_stored = y * 2^CY
VCOL = float(2.0 ** (LGQ - CY))          # ones-column value
EXPSCALE = (1.0 / np.sqrt(HD)) * (2.0 ** (-2 * LGQ))
ODESC = float(2.0 ** (-(CY + LGW)))      # final out descale


def _split_multi_waits(nc):
    """Move extra sync waits onto same-engine NoOps (1-wait ISA limit)."""
    for fn in nc.m.functions:
        for bb in fn.blocks:
            out = []
            for inst in bb.instructions:
                si = inst.sync_info
                if si is not None and si.on_wait and len(si.on_wait) > 1:
                    waits = list(si.on_wait)
                    for j, w in enumerate(waits[:-1]):
                        nop = mybir.InstNoOp(name=f"{inst.name}-wsp{j}")
                        nop.engine = inst.engine
                        nop.sync_info = mybir.SyncInfo(on_wait=[w], on_update=[])
                        out.append(nop)
                    inst.sync_info = mybir.SyncInfo(
                        on_wait=[waits[-1]], on_update=list(si.on_update)
                    )
                out.append(inst)
            bb.instructions = out


def _build():
    nc = bass.Bass("TRN2", target_bir_lowering=False, debug=False)
    # x^T hi/lo fp8, host-transposed: per seg [c(8), pl(2: lo,hi), q(512)]
    # fp8 = [128, 4096] bf16-typed; 4 segs concatenated.
    xT8d = nc.dram_tensor("xT8", [128, NSEG * 4096], BF16,
                          kind="ExternalInput").ap()
    # Q/K weights per group: [m(8), c(8), pl(2: hi,lo), 128] fp8 = [128, 8192]
    wqkd = [nc.dram_tensor(f"wqk8_{g}", [128, 8192], BF16,
                           kind="ExternalInput").ap() for g in range(2)]
    # V / proj weights: [c(8), pl(2: hi,lo), 1024] fp8 = [128, 8192]
    wvd = nc.dram_tensor("wv8", [128, 8192], BF16, kind="ExternalInput").ap()
    wpd = nc.dram_tensor("wp8", [128, 8192], BF16, kind="ExternalInput").ap()
    out = nc.dram_tensor("out", [TOK, C], F32, kind="ExternalOutput").ap()

    ident_d = nc.inline_tensor(np.eye(128, dtype=np.float32), "ident_c").ap()

    with tile.TileContext(nc) as tc:
        with (
            tc.tile_pool(name="const", bufs=1) as cpool,
            tc.tile_pool(name="wres", bufs=1) as wres,
            tc.tile_pool(name="work", bufs=1) as work,
            tc.tile_pool(name="ps", bufs=1, space="PSUM") as pspool,
        ):
            ps = pspool.tile([128, 4096], F32, tag="ps", name="ps")

            def bank(b):
                return ps[0:128, 512 * b:512 * (b + 1)]

            def bank_bf(b, j):
                return ps[0:128, 512 * b + 64 * j: 512 * b + 64 * (j + 1)].bitcast(BF16)

            # ---- constants
            identf = cpool.tile([128, 128], F32, tag="identf", name="identf")
            identb = cpool.tile([128, 128], BF16, tag="identb", name="identb")
            scratch = cpool.tile([128, 128], BF16, tag="scr", name="scr")
            nc.vector.memset(scratch[:], 1.0)
            # dummy transposes: keep PE busy (pstate ramp) while DMAs land
            for _ in range(64):
                nc.tensor.matmul(bank_bf(6, 0), scratch[:], scratch[:],
                                 is_transpose=True, start=True, stop=True,
                                 skip_group_check=True)

            # ---- resident weights (fp8 hi/lo planes in bf16-typed tiles)
            wqk = [wres.tile([128, 8192], BF16, tag=f"wqk{g}", name=f"wqk{g}")
                   for g in range(2)]
            wv = wres.tile([128, 8192], BF16, tag="wv", name="wv")
            wpj = wres.tile([128, 8192], BF16, tag="wpj", name="wpj")

            # fp8 4-dim views [p, c, pl, cols]
            def v4(t, cols):
                return t[:].bitcast(F8).rearrange(
                    "p (c pl m) -> p c pl m", c=8, pl=2)

            wqkv4 = [v4(wqk[g], 1024) for g in range(2)]
            wvv4 = v4(wv, 1024)
            wpv4 = v4(wpj, 1024)

            def wflat(t, c):
                # [p, pl(2), 1024] fp8 view of chunk c (pl: hi, lo)
                return t[:].bitcast(F8)[:, c * 2048:(c + 1) * 2048].rearrange(
                    "p (pl m) -> p pl m", pl=2)

            def load_weights(j0=0, j1=32, eng=None):
                # Q/K groups as m-column slices (contiguous src -> strided
                # dst) so prologue QKV tiles stream as slices land; V/proj
                # as plain chunk rows.
                eng = eng or nc.sync
                jobs = []
                for g in range(2):
                    for m in range(8):
                        dst = wqk[g][:].rearrange(
                            "p (c pl w) -> p c pl w", c=8, pl=2)[
                            :, :, :, m * 64:(m + 1) * 64]
                        jobs.append((wqkd[g][:, m * 1024:(m + 1) * 1024], dst))
                for cc in range(8):
                    jobs.append((wvd[:, cc * 1024:(cc + 1) * 1024],
                                 wv[:, cc * 1024:(cc + 1) * 1024]))
                for cc in range(8):
                    jobs.append((wpd[:, cc * 1024:(cc + 1) * 1024],
                                 wpj[:, cc * 1024:(cc + 1) * 1024]))
                for i, (src, dst) in list(enumerate(jobs))[j0:j1]:
                    eng.dma_start(dst, src)

            gb_state = [0]

            def next_gb():
                gb_state[0] ^= 1
                return 6 + gb_state[0]

            def x_load(s):
                t = work.tile([128, 4096], BF16, tag="xT", bufs=2,
                              name=f"xT8_{s}")
                nc.sync.dma_start(t[:], xT8d[:, s * 4096:(s + 1) * 4096])
                return t

            def xviews(xt):
                xf = xt[:].bitcast(F8)
                return (xf.rearrange("p (c pl q) -> p c pl q", c=8, pl=2), xf)

            def dr_group(bnk, lhs_main, rhs_main, lhs_cross, rhs_cross,
                         wn=512, w0=0):
                # 4 main insts (hi cc-pairs) + 8 cross insts, one psum group
                for j in range(4):
                    nc.tensor.matmul(bnk, lhs_main(j), rhs_main(j),
                                     start=(j == 0), stop=False, perf_mode=DR)
                for c in range(8):
                    nc.tensor.matmul(bnk, lhs_cross(c), rhs_cross(c),
                                     start=False, stop=(c == 7), perf_mode=DR)

            def qkv_closures(s, xt, qkt, vps):
                xv, xf = xviews(xt)
                cls = []
                for m in range(8):
                    for g in range(2):
                        def f(g=g, m=m):
                            b = next_gb()
                            dr_group(
                                bank(b),
                                lambda j, g=g, m=m: wqkv4[g][
                                    :, 2 * j:2 * j + 2, 0:1,
                                    m * 128:(m + 1) * 128],
                                lambda j: xv[:, 2 * j:2 * j + 2, 1:2, :],
                                lambda c, g=g, m=m: wflat(wqk[g], c)[
                                    :, :, m * 128:(m + 1) * 128],
                                lambda c: xf[:, c * 1024:(c + 1) * 1024]
                                .rearrange("p (pl q) -> p pl q", pl=2),
                            )
                            nc.vector.tensor_copy(
                                qkt[:, (g * 8 + m) * 512:(g * 8 + m + 1) * 512],
                                bank(b))
                        cls.append(f)
                for kt in range(4):
                    for vn in range(2):
                        def f(kt=kt, vn=vn):
                            b = next_gb()
                            dr_group(
                                bank(b),
                                lambda j, kt=kt: xv[
                                    :, 2 * j:2 * j + 2, 1:2,
                                    kt * 128:(kt + 1) * 128],
                                lambda j, vn=vn: wvv4[
                                    :, 2 * j:2 * j + 2, 0:1,
                                    vn * 512:(vn + 1) * 512],
                                lambda c, kt=kt: xf[:, c * 1024:(c + 1) * 1024]
                                .rearrange("p (pl q) -> p pl q", pl=2)
                                [:, :, kt * 128:(kt + 1) * 128],
                                lambda c, vn=vn: wflat(wv, c)[
                                    :, :, vn * 512:(vn + 1) * 512],
                            )
                            nc.vector.tensor_copy(
                                vps[kt].rearrange("p (h w) -> p h w", w=66)
                                [:, vn * 8:(vn + 1) * 8, 0:64],
                                bank(b).rearrange("p (h w) -> p h w", w=64))
                        cls.append(f)
                return cls

            def ytT8_views(yt8):
                f = yt8[:].bitcast(F8)
                return (f.rearrange("p (c pl q) -> p c pl q", c=8, pl=2), f)

            def proj_closures(s, yt8, obs, split_last=False):
                yv, yf = ytT8_views(yt8)
                cls = []
                for qt in range(4):
                    for vn in range(2):
                        def f(qt=qt, vn=vn):
                            widths = ([384, 128] if (split_last and qt == 3
                                                     and vn == 1) else [512])
                            w0 = vn * 512
                            for wn in widths:
                                b = next_gb()
                                dr_group(
                                    bank(b)[:, 0:wn],
                                    lambda j, qt=qt: yv[
                                        :, 2 * j:2 * j + 2, 1:2,
                                        qt * 128:(qt + 1) * 128],
                                    lambda j, w0=w0, wn=wn: wpv4[
                                        :, 2 * j:2 * j + 2, 0:1, w0:w0 + wn],
                                    lambda c, qt=qt: yf[
                                        :, c * 1024:(c + 1) * 1024]
                                    .rearrange("p (pl q) -> p pl q", pl=2)
                                    [:, :, qt * 128:(qt + 1) * 128],
                                    lambda c, w0=w0, wn=wn: wflat(wpj, c)[
                                        :, :, w0:w0 + wn],
                                )
                                nc.vector.tensor_scalar_mul(
                                    obs[qt][:, w0:w0 + wn], bank(b)[:, 0:wn],
                                    ODESC)
                                nc.sync.dma_start(
                                    out[s * SEG + qt * 128:
                                        s * SEG + (qt + 1) * 128,
                                        w0:w0 + wn],
                                    obs[qt][:, w0:w0 + wn])
                                w0 += wn
                        cls.append(f)
                return cls

            # ---------------- attention pieces (bf16) ----------
            # S^T double-buffered over 4 banks (part0 -> 0,1; part1 -> 2,3)
            # so st_part(h+1) never WAR-waits on exp(h); A@V packs all 4 qt
            # groups into one bank (4/5 by head parity), freeing banks 2,3.
            def st_part(qkt, h, part):
                r0 = 64 * (h % 2)
                qrow = qkt[r0:r0 + 64, (h // 2) * 512:(h // 2) * 512 + 512]
                for i in range(2):
                    kt = 2 * part + i
                    nc.tensor.matmul(
                        bank(kt),
                        qkt[r0:r0 + 64,
                            (8 + h // 2) * 512 + kt * 128:
                            (8 + h // 2) * 512 + (kt + 1) * 128],
                        qrow, start=True, stop=True)

            def exp_part(s, h, at0, part):
                nc.scalar.activation(
                    at0[:, part * 1024:(part + 1) * 1024],
                    ps[0:128, part * 1024:(part + 1) * 1024], EXP,
                    scale=EXPSCALE)

            def av_bank(h):
                return 4 + (h % 2)

            def av_head(s, h, at0, vps):
                B = 512 * av_bank(h)
                for qt in range(4):
                    for kt in range(4):
                        nc.tensor.matmul(
                            ps[0:128, B + 65 * qt: B + 65 * qt + 65],
                            at0[:, kt * 512 + qt * 128: kt * 512 + (qt + 1) * 128],
                            vps[kt][:, h * 66: h * 66 + 65],
                            start=(kt == 0), stop=(kt == 3))
                ostg = work.tile([128, 260], F32, tag="ostg", bufs=2,
                                 name=f"ostg{s}_{h}")
                nc.vector.tensor_copy(ostg[:], ps[0:128, B:B + 260])
                rz = work.tile([128, 4], F32, tag="rz", bufs=2, name=f"rz{s}_{h}")
                nc.vector.reciprocal(
                    rz[:].rearrange("p (q w) -> p q w", w=1),
                    ostg[:].rearrange("p (q w) -> p q w", w=65)[:, :, 64:65])
                return ostg, rz

            def scales_head(s, h, ostg, rz, yts):
                for qt in range(4):
                    nc.vector.tensor_scalar_mul(
                        yts[qt][:, h * 64:(h + 1) * 64],
                        ostg[:, qt * 65: qt * 65 + 64],
                        rz[:, qt:qt + 1])

            # ---------------- build the pipeline ----------------
            xts = [None] * NSEG
            qkts = [None] * NSEG
            vpss = [None] * NSEG
            yts = [None] * NSEG
            obs = [None] * NSEG

            def make_seg_tiles(s):
                qkts[s] = work.tile([128, 16 * 512], BF16, tag="qkt", bufs=2,
                                    name=f"qkt{s}")
                vpss[s] = [work.tile([128, 16 * 66], BF16, tag=f"vp{kt}", bufs=2,
                                     name=f"vp{s}_{kt}") for kt in range(4)]
                yts[s] = [work.tile([128, C], BF16, tag=f"yt{qt}", bufs=2,
                                    name=f"yt{s}_{qt}") for qt in range(4)]
                obs[s] = [work.tile([128, C], F32, tag=f"ob{qt}", bufs=1,
                                    name=f"ob{s}_{qt}") for qt in range(4)]
                for kt in range(4):
                    nc.vector.memset(
                        vpss[s][kt].rearrange("p (h w) -> p h w", w=66)[:, :, 64:65],
                        VCOL)

            def yt_chunk_pe(yt_tiles, yt8, c, b):
                # PE-transpose Y^T chunk c (4 qt tiles of [128,128] bf16)
                # into bank b (the just-drained AV parity bank), then DVE
                # hi/lo fp8 conversion straight from PSUM -- conversion
                # input is always ready (no DMA latency), so it never
                # head-blocks the DVE stream.
                for qt in range(4):
                    nc.tensor.transpose(
                        bank_bf(b, qt),
                        yt_tiles[qt][:, c * 128:(c + 1) * 128], identb[:])
                src = ps[0:128, 512 * b:512 * b + 256].bitcast(BF16)
                s3 = src.rearrange("p (c q) -> p c q", c=1)
                yb = yt8[:].bitcast(F8).rearrange("p (c b) -> p c b", b=1024)
                hi = yb[:, c:c + 1, 512:1024]
                lo = yb[:, c:c + 1, 0:512]
                nc.vector.tensor_copy(hi, s3)
                nc.vector.tensor_tensor(lo, s3, hi, mybir.AluOpType.subtract)

            # XBAR path for chunks 0-3 (latency fully hidden: issue at h8,
            # convert at h12) -- saves the PE transpose cycles there
            yt03_stage = work.tile([128, 2048], BF16, tag="yt03", bufs=1,
                                   name="yt03_stage")

            def yt03_xpose(yt_tiles):
                for qt in range(4):
                    nc.sync.dma_start_transpose(
                        yt03_stage[:].rearrange("p (c q) -> p c q", q=512)
                        [:, :, qt * 128:(qt + 1) * 128],
                        yt_tiles[qt][:, 0:512])

            def yt03_convert(yt8):
                yb = yt8[:].bitcast(F8).rearrange("p (c b) -> p c b", b=1024)
                hi = yb[:, 0:4, 512:1024]
                lo = yb[:, 0:4, 0:512]
                src = yt03_stage[:].rearrange("p (c q) -> p c q", q=512)
                nc.vector.tensor_copy(hi, src)
                nc.vector.tensor_tensor(lo, src, hi, mybir.AluOpType.subtract)

            # prologue
            xts[0] = x_load(0)
            nc.sync.dma_start(identf[:], ident_d[:, :])
            nc.vector.tensor_copy(identb[:], identf[:])
            load_weights(0, 16)    # Q then K m-slices
            load_weights(16, 24)   # wv
            xts[1] = x_load(1)
            load_weights(24, 32)   # wp
            _mark(nc, "prologue-loads")
            make_seg_tiles(0)
            qc0 = qkv_closures(0, xts[0], qkts[0], vpss[0])
            for f in qc0[0::2][:8]:   # Q tiles
                f()
            for f in qc0[1::2][:8]:   # K tiles
                f()
            for f in qc0[16:24:2]:    # V vn=0 tiles (needed from AV(0))
                f()
            # V vn=1 (needed only from AV(8)): front of round 0's worklist
            prologue_leftover = list(qc0[17:24:2])
            _mark(nc, "prologue-qkv0")

            deferred_qkv = []
            deferred_proj = []
            yt8s = [None] * NSEG
            for r in range(NSEG):
                worklist = list(prologue_leftover) + list(deferred_qkv)
                prologue_leftover = []
                deferred_qkv = []
                # deferred proj first: they read a yt8/obs generation that
                # this round's conversions/projections will overwrite
                worklist += deferred_proj
                deferred_proj = []
                if r + 1 < NSEG:
                    make_seg_tiles(r + 1)
                    qc = qkv_closures(r + 1, xts[r + 1], qkts[r + 1],
                                      vpss[r + 1])
                    if r + 1 == NSEG - 1:
                        # defer Q/K m2-m5 and V vn=1 to the last round,
                        # which otherwise starves the PE in its tail
                        worklist += qc[0:4] + qc[12:16] + qc[16:24:2]
                        deferred_qkv = qc[4:12] + qc[17:24:2]
                    else:
                        worklist += qc
                # yt8 for THIS round's segment: filled by chunked XBAR +
                # conversions as heads complete (input always landed when
                # the DVE instruction issues -- no in-order DVE blockage)
                yt8s[r] = work.tile([128, 4096], BF16, tag="yt8", bufs=2,
                                    name=f"yt8_{r}")
                if r >= 1:
                    pc = proj_closures(r - 1, yt8s[r - 1], obs[r - 1])
                    if r < NSEG - 1:
                        worklist += pc[0:4]
                        deferred_proj = pc[4:8]
                    else:
                        worklist += pc
                _mark(nc, f"r{r}-startbatch")

                qkt, vps = qkts[r], vpss[r]
                at0s = {}
                wi = 0
                NPOP = 31 if r == NSEG - 1 else 34
                ci = [0]

                def pop_work():
                    nonlocal wi
                    ci[0] += 1
                    W = len(worklist)
                    target = (W * ci[0]) // NPOP
                    while wi < min(target, W):
                        worklist[wi]()
                        wi += 1

                last = r == NSEG - 1
                yt8_3 = yt8s[r]

                def proj3_A():
                    # first contraction half (cc 0-3) of proj(3)
                    yv3, yf3 = ytT8_views(yt8_3)
                    cls = []
                    for qt in range(4):
                        for vn in range(2):
                            def f(qt=qt, vn=vn):
                                b = next_gb()
                                for j in range(2):
                                    nc.tensor.matmul(
                                        bank(b),
                                        yv3[:, 2 * j:2 * j + 2, 1:2,
                                            qt * 128:(qt + 1) * 128],
                                        wpv4[:, 2 * j:2 * j + 2, 0:1,
                                             vn * 512:(vn + 1) * 512],
                                        start=(j == 0), stop=False,
                                        perf_mode=DR)
                                for c in range(4):
                                    nc.tensor.matmul(
                                        bank(b),
                                        yf3[:, c * 1024:(c + 1) * 1024]
                                        .rearrange("p (pl q) -> p pl q", pl=2)
                                        [:, :, qt * 128:(qt + 1) * 128],
                                        wflat(wpj, c)[
                                            :, :, vn * 512:(vn + 1) * 512],
                                        start=False, stop=(c == 3),
                                        perf_mode=DR)
                                nc.vector.tensor_scalar_mul(
                                    obs[3][qt][:, vn * 512:(vn + 1) * 512],
                                    bank(b), ODESC)
                            cls.append(f)
                    return cls

                for h in range(16):
                    at0s[h] = work.tile([128, 2048], BF16, tag="at0", bufs=3,
                                        name=f"at0_{r}_{h}")
                    st_part(qkt, h, 0)
                    exp_part(r, h, at0s[h], 0)
                    pop_work()
                    st_part(qkt, h, 1)
                    exp_part(r, h, at0s[h], 1)
                    if h >= 1:
                        ostg, rz = av_head(r, h - 1, at0s[h - 1], vps)
                        scales_head(r, h - 1, ostg, rz, yts[r])
                        del at0s[h - 1]
                        if h == 8:
                            yt03_xpose(yts[r])
                        if h >= 11 and h % 2 == 1:
                            # chunk (h-3)/2 (4..6) complete: PE transpose +
                            # convert into the AV parity bank (AV(h) claims
                            # it only at h+1; AV(h-2)'s drain already done)
                            yt_chunk_pe(yts[r], yt8s[r], (h - 3) // 2,
                                        av_bank(h))
                    if h == 13:
                        yt03_convert(yt8s[r])
                        if last:
                            worklist.extend(proj3_A())
                    pop_work()
                    _mark(nc, f"r{r}-h{h}")
                if last:
                    # kt0/kt1 accumulation can run while exp1(15) computes.
                    # One bank per qt: hardware start=True arms the whole
                    # 2KB zero region, so interleaved open groups must not
                    # share a bank.  Banks 0,1 are free (exp0(15) done) and
                    # 4,5 (AV parity pair).
                    b15 = [0, 1, 2, 3]
                    for qt in range(4):
                        for kt in range(2):
                            nc.tensor.matmul(
                                bank(b15[qt])[:, 0:65],
                                at0s[15][:, kt * 512 + qt * 128:
                                         kt * 512 + (qt + 1) * 128],
                                vps[kt][:, 15 * 66: 15 * 66 + 65],
                                start=(kt == 0), stop=False)
                    for qt in range(4):
                        for kt in range(2, 4):
                            nc.tensor.matmul(
                                bank(b15[qt])[:, 0:65],
                                at0s[15][:, kt * 512 + qt * 128:
                                         kt * 512 + (qt + 1) * 128],
                                vps[kt][:, 15 * 66: 15 * 66 + 65],
                                start=False, stop=(kt == 3))
                    rz = work.tile([128, 4], F32, tag="rz", bufs=2,
                                   name=f"rz{r}_15")
                    for qt in range(4):
                        nc.vector.reciprocal(
                            rz[:, qt:qt + 1],
                            bank(b15[qt])[:, 64:65])
                    for qt in range(4):
                        nc.vector.tensor_scalar_mul(
                            yts[r][qt][:, 15 * 64:16 * 64],
                            bank(b15[qt])[:, 0:64],
                            rz[:, qt:qt + 1])
                else:
                    ostg, rz = av_head(r, 15, at0s[15], vps)
                    scales_head(r, 15, ostg, rz, yts[r])
                yt_chunk_pe(yts[r], yt8s[r], 7, av_bank(14))
                if r + 2 < NSEG:
                    # late x load: consumers pop early next round; issuing
                    # here keeps the SP DMA queue clear for the yt03 XBAR
                    xts[r + 2] = x_load(r + 2)
                ci[0] = NPOP - 1
                pop_work()

            # epilogue: proj(3) second contraction half (cc 4-7) accumulates
            # into obs with add + descale (Y^T chunks all converted in-round).
            _mark(nc, "rounds-done")
            yv3c, yf3c = ytT8_views(yt8_3)
            bi = 0
            for qt in range(4):
                for vn in range(2):
                    widths = [384, 128] if (qt == 3 and vn == 1) else [512]
                    w0 = vn * 512
                    for wn in widths:
                        b = [4, 5, 6, 7][bi % 4]
                        bi += 1
                        for j in range(2, 4):
                            nc.tensor.matmul(
                                bank(b)[:, 0:wn],
                                yv3c[:, 2 * j:2 * j + 2, 1:2,
                                     qt * 128:(qt + 1) * 128],
                                wpv4[:, 2 * j:2 * j + 2, 0:1, w0:w0 + wn],
                                start=(j == 2), stop=False, perf_mode=DR)
                        for c in range(4, 8):
                            nc.tensor.matmul(
                                bank(b)[:, 0:wn],
                                yf3c[:, c * 1024:(c + 1) * 1024]
                                .rearrange("p (pl q) -> p pl q", pl=2)
                                [:, :, qt * 128:(qt + 1) * 128],
                                wflat(wpj, c)[:, :, w0:w0 + wn],
                                start=False, stop=(c == 7), perf_mode=DR)
                        nc.vector.scalar_tensor_tensor(
                            obs[3][qt][:, w0:w0 + wn],
                            bank(b)[:, 0:wn], ODESC,
                            obs[3][qt][:, w0:w0 + wn],
                            mybir.AluOpType.mult, mybir.AluOpType.add)
                        nc.sync.dma_start(
                            out[3 * SEG + qt * 128: 3 * SEG + (qt + 1) * 128,
                                w0:w0 + wn],
                            obs[3][qt][:, w0:w0 + wn])
                        w0 += wn

    _split_multi_waits(nc)
    return nc


_NC = None


def _hilo(t, scale):
    import ml_dtypes
    FP8 = ml_dtypes.float8_e4m3
    ts = (t.astype(np.float32) * scale)
    hi = ts.astype(FP8)
    lo = (ts - hi.astype(np.float32)).astype(FP8)
    return hi, lo


def _prep_inputs(x, w_attn, w_proj):
    """Host-side: scale, hi/lo fp8 split, transpose/layout packing."""
    import ml_dtypes
    BF = ml_dtypes.bfloat16
    x = np.asarray(x, dtype=np.float32)
    wa = np.asarray(w_attn, dtype=np.float32)
    wp = np.asarray(w_proj, dtype=np.float32)

    # x^T per core: [p, seg, c, pl(lo,hi), q] fp8 -> [128, NSEG*4096] bf16
    xh, xl = _hilo(x, 2.0 ** LGX)          # [T, C]
    def xt_layout(a):
        # [T, C] -> [128p, T//SEG seg, 8c, SEG q]
        return a.reshape(T // SEG, SEG, 8, 128).transpose(3, 0, 2, 1)
    xs = np.stack([xt_layout(xl), xt_layout(xh)], axis=3)
    # [128, nseg_total, c, pl, q] -> bytes -> bf16 cols
    xs = np.ascontiguousarray(xs).reshape(128, -1).view(np.uint16).view(BF)

    # Q/K groups: [m, c, pl(hi,lo), 128j] fp8 per partition
    wqk8 = []
    for g in range(2):
        wh, wl = _hilo(wa[:, g * C:(g + 1) * C], 2.0 ** LGW)
        def wl_layout(a):
            # [K=1024, N=1024] -> [128p, 8m, 8c, 128j]
            return a.reshape(8, 128, 8, 128).transpose(1, 2, 0, 3)
        # [p, m, c, pl, j] with pl (hi, lo)
        ws = np.stack([wl_layout(wh), wl_layout(wl)], axis=3)
        ws = np.ascontiguousarray(ws)
        wqk8.append(ws.reshape(128, -1).view(np.uint16).view(BF))

    def cpl_layout(w):
        # [K=1024, N=1024] -> [c, pl(hi,lo), n] per partition
        wh, wl = _hilo(w, 2.0 ** LGW)
        def lay(a):
            return a.reshape(8, 128, C).transpose(1, 0, 2)   # [p, c, n]
        ws = np.stack([lay(wh), lay(wl)], axis=2)            # [p, c, pl, n]
        return np.ascontiguousarray(ws).reshape(128, -1).view(np.uint16).view(BF)

    wv8 = cpl_layout(wa[:, 2 * C:3 * C])
    wp8 = cpl_layout(wp)
    return xs, wqk8, wv8, wp8


def kernel(x, w_attn, w_proj, split_sections):
    global _NC
    if _NC is None:
        _NC = _build()
    xs, wqk8, wv8, wp8 = _prep_inputs(x, w_attn, w_proj)
    # xs: [128, (T//SEG)*4096] bf16; per core slice NSEG segments
    in_maps = [
        {"xT8": np.ascontiguousarray(
            xs[:, i * NSEG * 4096:(i + 1) * NSEG * 4096]),
         "wqk8_0": wqk8[0], "wqk8_1": wqk8[1], "wv8": wv8, "wp8": wp8}
        for i in range(NCORES)
    ]
    res = run_bass_kernel_spmd(_NC, in_maps, core_ids=list(range(NCORES)))
    return np.concatenate([res.results[i]["out"] for i in range(NCORES)], axis=0)


if __name__ == "__main__":
    rng = np.random.default_rng(0)
    x = rng.standard_normal((T, C), dtype=np.float32)
    wa = (rng.standard_normal((C, 3 * C), dtype=np.float32) / np.sqrt(C)).astype(np.float32)
    wpj = (rng.standard_normal((C, C), dtype=np.float32) / np.sqrt(C)).astype(np.float32)
    y = kernel(x, wa, wpj, np.arange(1, 32) * 512)
    print("out", y.shape, y.dtype, np.abs(y).mean())


# revision 80
# speedup vs baseline: 1.0057x; 1.0057x over previous
"""MHSA over 32 independent 512-token segments, segment-parallel across 8
NeuronCores (4 segments / 2048 tokens per core, zero cross-core traffic).

QKV and output projection run as fp8e4m3 DoubleRow matmuls (0.5 cyc/row in
the PE) with hi+lo error compensation: every operand X is split host-side
into X = Xh + Xl (two fp8 planes, planar layout so the ldweights k-tile
stride meets the dual-fp8 ISA rule step%16==0), and X@W is computed as
Xh@Wh + (Xl@Wh + Xh@Wl), dropping only the lo*lo term.  Operands are
pre-scaled by powers of two (x*2^4, w*2^9) to center them in e4m3 range;
the scales are folded into the exp() argument, the A@V ones-column, and a
final tensor_scalar descale -- net precision is slightly BETTER than bf16.
A K=1024 output tile costs 12 DR insts x 256 cyc = 3072 cyc vs bf16's
8 x 512 = 4096.  x additionally arrives pre-transposed from the host
(x^T hi/lo planes), removing all on-device x transposes.

S = QK^T and A@V stay bf16 (contraction is 64/65-wide there; DoubleRow
with compensation degenerates to bf16 cost).  Per segment s, per head h:
  Q^T,K^T   DR fp8: lhsT=Wqk chunk, rhs=xT8          16x [128, 512]
  V         DR fp8, natural [tok, 1024] + s-col per head (A@V rowsum)
  S^T       = K^T' Q^T per head, 4x [64c -> 128k, 512q] into PSUM b0/b1
  A^T       = exp(S^T * 2^-26 / 8) bf16
  O'        = A^T.T @ [V_h|s]  natural [128q, 65] per qt -> bank 2+qt
  yt        = O'[:, 0:64] * recip(O'[:, 64]) per-partition scale (DVE)
  ytT8      = XBAR-transpose yt -> DVE hi/lo fp8 planes
  out       = ytT8 @ Wp8 (DR fp8) * 2^-13

PSUM banks: 0-1 S^T (ping-pong around exp), 2-5 A@V per qt, 6-7 QKV/proj
accumulation tiles.  3-stage software pipeline in PE emission order as in
the bf16 baseline (floor-paced worklist between attention heads).
"""

import numpy as np

import concourse.bass as bass
import concourse.mybir as mybir
import concourse.tile as tile
from concourse.bass_utils import run_bass_kernel_spmd

F32 = mybir.dt.float32
BF16 = mybir.dt.bfloat16
F8 = mybir.dt.float8e4
EXP = mybir.ActivationFunctionType.Exp
COPY = mybir.ActivationFunctionType.Copy
DR = mybir.MatmulPerfMode.DoubleRow

PHASE_MARKS = []


def _mark(nc, label):
    insts = list(nc.all_instructions())
    last = insts[-1].name if insts else "I-0"
    PHASE_MARKS.append((label, int(last.split("-")[1])))


T, C, H, HD = 16384, 1024, 16, 64
NCORES = 8
TOK = T // NCORES          # 2048 tokens per core
SEG = 512                  # tokens per segment
NSEG = TOK // SEG          # 4 segments per core
LGX, LGW = 4, 9            # x scaled by 2^4, weights by 2^9
LGQ = LGX + LGW            # Q/K/V scale 2^13
CY = 4                     # y_stored = y * 2^CY
VCOL = float(2.0 ** (LGQ - CY))          # ones-column value
EXPSCALE = (1.0 / np.sqrt(HD)) * (2.0 ** (-2 * LGQ))
ODESC = float(2.0 ** (-(CY + LGW)))      # final out descale


def _split_multi_waits(nc):
    """Move extra sync waits onto same-engine NoOps (1-wait ISA limit)."""
    for fn in nc.m.functions:
        for bb in fn.blocks:
            out = []
            for inst in bb.instructions:
                si = inst.sync_info
                if si is not None and si.on_wait and len(si.on_wait) > 1:
                    waits = list(si.on_wait)
                    for j, w in enumerate(waits[:-1]):
                        nop = mybir.InstNoOp(name=f"{inst.name}-wsp{j}")
                        nop.engine = inst.engine
                        nop.sync_info = mybir.SyncInfo(on_wait=[w], on_update=[])
                        out.append(nop)
                    inst.sync_info = mybir.SyncInfo(
                        on_wait=[waits[-1]], on_update=list(si.on_update)
                    )
                out.append(inst)
            bb.instructions = out


def _build():
    nc = bass.Bass("TRN2", target_bir_lowering=False, debug=False)
    # x^T hi/lo fp8, host-transposed: per seg [c(8), pl(2: lo,hi), q(512)]
    # fp8 = [128, 4096] bf16-typed; 4 segs concatenated.
    xT8d = nc.dram_tensor("xT8", [128, NSEG * 4096], BF16,
                          kind="ExternalInput").ap()
    # Q/K weights per group: [m(8), c(8), pl(2: hi,lo), 128] fp8 = [128, 8192]
    wqkd = [nc.dram_tensor(f"wqk8_{g}", [128, 8192], BF16,
                           kind="ExternalInput").ap() for g in range(2)]
    # V / proj weights: [c(8), pl(2: hi,lo), 1024] fp8 = [128, 8192]
    wvd = nc.dram_tensor("wv8", [128, 8192], BF16, kind="ExternalInput").ap()
    wpd = nc.dram_tensor("wp8", [128, 8192], BF16, kind="ExternalInput").ap()
    out = nc.dram_tensor("out", [TOK, C], F32, kind="ExternalOutput").ap()

    ident_d = nc.inline_tensor(np.eye(128, dtype=np.float32), "ident_c").ap()

    with tile.TileContext(nc) as tc:
        with (
            tc.tile_pool(name="const", bufs=1) as cpool,
            tc.tile_pool(name="wres", bufs=1) as wres,
            tc.tile_pool(name="work", bufs=1) as work,
            tc.tile_pool(name="ps", bufs=1, space="PSUM") as pspool,
        ):
            ps = pspool.tile([128, 4096], F32, tag="ps", name="ps")

            def bank(b):
                return ps[0:128, 512 * b:512 * (b + 1)]

            def bank_bf(b, j):
                return ps[0:128, 512 * b + 64 * j: 512 * b + 64 * (j + 1)].bitcast(BF16)

            # ---- constants
            identf = cpool.tile([128, 128], F32, tag="identf", name="identf")
            identb = cpool.tile([128, 128], BF16, tag="identb", name="identb")
            scratch = cpool.tile([128, 128], BF16, tag="scr", name="scr")
            nc.vector.memset(scratch[:], 1.0)
            # dummy transposes: keep PE busy (pstate ramp) while DMAs land
            for _ in range(64):
                nc.tensor.matmul(bank_bf(6, 0), scratch[:], scratch[:],
                                 is_transpose=True, start=True, stop=True,
                                 skip_group_check=True)

            # ---- resident weights (fp8 hi/lo planes in bf16-typed tiles)
            wqk = [wres.tile([128, 8192], BF16, tag=f"wqk{g}", name=f"wqk{g}")
                   for g in range(2)]
            wv = wres.tile([128, 8192], BF16, tag="wv", name="wv")
            wpj = wres.tile([128, 8192], BF16, tag="wpj", name="wpj")

            # fp8 4-dim views [p, c, pl, cols]
            def v4(t, cols):
                return t[:].bitcast(F8).rearrange(
                    "p (c pl m) -> p c pl m", c=8, pl=2)

            wqkv4 = [v4(wqk[g], 1024) for g in range(2)]
            wvv4 = v4(wv, 1024)
            wpv4 = v4(wpj, 1024)

            def wflat(t, c):
                # [p, pl(2), 1024] fp8 view of chunk c (pl: hi, lo)
                return t[:].bitcast(F8)[:, c * 2048:(c + 1) * 2048].rearrange(
                    "p (pl m) -> p pl m", pl=2)

            def load_weights(j0=0, j1=32, eng=None):
                # Q/K groups as m-column slices (contiguous src -> strided
                # dst) so prologue QKV tiles stream as slices land; V/proj
                # as plain chunk rows.
                eng = eng or nc.sync
                jobs = []
                for g in range(2):
                    for m in range(8):
                        dst = wqk[g][:].rearrange(
                            "p (c pl w) -> p c pl w", c=8, pl=2)[
                            :, :, :, m * 64:(m + 1) * 64]
                        jobs.append((wqkd[g][:, m * 1024:(m + 1) * 1024], dst))
                for cc in range(8):
                    jobs.append((wvd[:, cc * 1024:(cc + 1) * 1024],
                                 wv[:, cc * 1024:(cc + 1) * 1024]))
                for cc in range(8):
                    jobs.append((wpd[:, cc * 1024:(cc + 1) * 1024],
                                 wpj[:, cc * 1024:(cc + 1) * 1024]))
                for i, (src, dst) in list(enumerate(jobs))[j0:j1]:
                    eng.dma_start(dst, src)

            gb_state = [0]

            def next_gb():
                gb_state[0] ^= 1
                return 6 + gb_state[0]

            def x_load(s):
                t = work.tile([128, 4096], BF16, tag="xT", bufs=2,
                              name=f"xT8_{s}")
                nc.sync.dma_start(t[:], xT8d[:, s * 4096:(s + 1) * 4096])
                return t

            def xviews(xt):
                xf = xt[:].bitcast(F8)
                return (xf.rearrange("p (c pl q) -> p c pl q", c=8, pl=2), xf)

            def dr_group(bnk, lhs_main, rhs_main, lhs_cross, rhs_cross,
                         wn=512, w0=0):
                # 4 main insts (hi cc-pairs) + 8 cross insts, one psum group
                for j in range(4):
                    nc.tensor.matmul(bnk, lhs_main(j), rhs_main(j),
                                     start=(j == 0), stop=False, perf_mode=DR)
                for c in range(8):
                    nc.tensor.matmul(bnk, lhs_cross(c), rhs_cross(c),
                                     start=False, stop=(c == 7), perf_mode=DR)

            def qkv_closures(s, xt, qkt, vps):
                xv, xf = xviews(xt)
                cls = []
                for m in range(8):
                    for g in range(2):
                        def f(g=g, m=m):
                            b = next_gb()
                            dr_group(
                                bank(b),
                                lambda j, g=g, m=m: wqkv4[g][
                                    :, 2 * j:2 * j + 2, 0:1,
                                    m * 128:(m + 1) * 128],
                                lambda j: xv[:, 2 * j:2 * j + 2, 1:2, :],
                                lambda c, g=g, m=m: wflat(wqk[g], c)[
                                    :, :, m * 128:(m + 1) * 128],
                                lambda c: xf[:, c * 1024:(c + 1) * 1024]
                                .rearrange("p (pl q) -> p pl q", pl=2),
                            )
                            nc.vector.tensor_copy(
                                qkt[:, (g * 8 + m) * 512:(g * 8 + m + 1) * 512],
                                bank(b))
                        cls.append(f)
                for kt in range(4):
                    for vn in range(2):
                        def f(kt=kt, vn=vn):
                            b = next_gb()
                            dr_group(
                                bank(b),
                                lambda j, kt=kt: xv[
                                    :, 2 * j:2 * j + 2, 1:2,
                                    kt * 128:(kt + 1) * 128],
                                lambda j, vn=vn: wvv4[
                                    :, 2 * j:2 * j + 2, 0:1,
                                    vn * 512:(vn + 1) * 512],
                                lambda c, kt=kt: xf[:, c * 1024:(c + 1) * 1024]
                                .rearrange("p (pl q) -> p pl q", pl=2)
                                [:, :, kt * 128:(kt + 1) * 128],
                                lambda c, vn=vn: wflat(wv, c)[
                                    :, :, vn * 512:(vn + 1) * 512],
                            )
                            nc.vector.tensor_copy(
                                vps[kt].rearrange("p (h w) -> p h w", w=66)
                                [:, vn * 8:(vn + 1) * 8, 0:64],
                                bank(b).rearrange("p (h w) -> p h w", w=64))
                        cls.append(f)
                return cls

            def ytT8_views(yt8):
                f = yt8[:].bitcast(F8)
                return (f.rearrange("p (c pl q) -> p c pl q", c=8, pl=2), f)

            def proj_closures(s, yt8, obs, split_last=False):
                yv, yf = ytT8_views(yt8)
                cls = []
                for qt in range(4):
                    for vn in range(2):
                        def f(qt=qt, vn=vn):
                            widths = ([384, 128] if (split_last and qt == 3
                                                     and vn == 1) else [512])
                            w0 = vn * 512
                            for wn in widths:
                                b = next_gb()
                                dr_group(
                                    bank(b)[:, 0:wn],
                                    lambda j, qt=qt: yv[
                                        :, 2 * j:2 * j + 2, 1:2,
                                        qt * 128:(qt + 1) * 128],
                                    lambda j, w0=w0, wn=wn: wpv4[
                                        :, 2 * j:2 * j + 2, 0:1, w0:w0 + wn],
                                    lambda c, qt=qt: yf[
                                        :, c * 1024:(c + 1) * 1024]
                                    .rearrange("p (pl q) -> p pl q", pl=2)
                                    [:, :, qt * 128:(qt + 1) * 128],
                                    lambda c, w0=w0, wn=wn: wflat(wpj, c)[
                                        :, :, w0:w0 + wn],
                                )
                                nc.vector.tensor_scalar_mul(
                                    obs[qt][:, w0:w0 + wn], bank(b)[:, 0:wn],
                                    ODESC)
                                nc.sync.dma_start(
                                    out[s * SEG + qt * 128:
                                        s * SEG + (qt + 1) * 128,
                                        w0:w0 + wn],
                                    obs[qt][:, w0:w0 + wn])
                                w0 += wn
                        cls.append(f)
                return cls

            # ---------------- attention pieces (bf16) ----------
            # S^T double-buffered over 4 banks (part0 -> 0,1; part1 -> 2,3)
            # so st_part(h+1) never WAR-waits on exp(h); A@V packs all 4 qt
            # groups into one bank (4/5 by head parity), freeing banks 2,3.
            def st_part(qkt, h, part):
                r0 = 64 * (h % 2)
                qrow = qkt[r0:r0 + 64, (h // 2) * 512:(h // 2) * 512 + 512]
                for i in range(2):
                    kt = 2 * part + i
                    nc.tensor.matmul(
                        bank(kt),
                        qkt[r0:r0 + 64,
                            (8 + h // 2) * 512 + kt * 128:
                            (8 + h // 2) * 512 + (kt + 1) * 128],
                        qrow, start=True, stop=True)

            def exp_part(s, h, at0, part):
                nc.scalar.activation(
                    at0[:, part * 1024:(part + 1) * 1024],
                    ps[0:128, part * 1024:(part + 1) * 1024], EXP,
                    scale=EXPSCALE)

            def av_bank(h):
                return 4 + (h % 2)

            def av_head(s, h, at0, vps):
                B = 512 * av_bank(h)
                for qt in range(4):
                    for kt in range(4):
                        nc.tensor.matmul(
                            ps[0:128, B + 65 * qt: B + 65 * qt + 65],
                            at0[:, kt * 512 + qt * 128: kt * 512 + (qt + 1) * 128],
                            vps[kt][:, h * 66: h * 66 + 65],
                            start=(kt == 0), stop=(kt == 3))
                ostg = work.tile([128, 260], F32, tag="ostg", bufs=2,
                                 name=f"ostg{s}_{h}")
                nc.vector.tensor_copy(ostg[:], ps[0:128, B:B + 260])
                rz = work.tile([128, 4], F32, tag="rz", bufs=2, name=f"rz{s}_{h}")
                nc.vector.reciprocal(
                    rz[:].rearrange("p (q w) -> p q w", w=1),
                    ostg[:].rearrange("p (q w) -> p q w", w=65)[:, :, 64:65])
                return ostg, rz

            def scales_head(s, h, ostg, rz, yts):
                for qt in range(4):
                    nc.vector.tensor_scalar_mul(
                        yts[qt][:, h * 64:(h + 1) * 64],
                        ostg[:, qt * 65: qt * 65 + 64],
                        rz[:, qt:qt + 1])

            # ---------------- build the pipeline ----------------
            xts = [None] * NSEG
            qkts = [None] * NSEG
            vpss = [None] * NSEG
            yts = [None] * NSEG
            obs = [None] * NSEG

            def make_seg_tiles(s):
                qkts[s] = work.tile([128, 16 * 512], BF16, tag="qkt", bufs=2,
                                    name=f"qkt{s}")
                vpss[s] = [work.tile([128, 16 * 66], BF16, tag=f"vp{kt}", bufs=2,
                                     name=f"vp{s}_{kt}") for kt in range(4)]
                yts[s] = [work.tile([128, C], BF16, tag=f"yt{qt}", bufs=2,
                                    name=f"yt{s}_{qt}") for qt in range(4)]
                obs[s] = [work.tile([128, C], F32, tag=f"ob{qt}", bufs=1,
                                    name=f"ob{s}_{qt}") for qt in range(4)]
                for kt in range(4):
                    nc.vector.memset(
                        vpss[s][kt].rearrange("p (h w) -> p h w", w=66)[:, :, 64:65],
                        VCOL)

            def yt_chunk_pe(yt_tiles, yt8, c, b):
                # PE-transpose Y^T chunk c (4 qt tiles of [128,128] bf16)
                # into bank b (the just-drained AV parity bank), then DVE
                # hi/lo fp8 conversion straight from PSUM -- conversion
                # input is always ready (no DMA latency), so it never
                # head-blocks the DVE stream.
                for qt in range(4):
                    nc.tensor.transpose(
                        bank_bf(b, qt),
                        yt_tiles[qt][:, c * 128:(c + 1) * 128], identb[:])
                src = ps[0:128, 512 * b:512 * b + 256].bitcast(BF16)
                s3 = src.rearrange("p (c q) -> p c q", c=1)
                yb = yt8[:].bitcast(F8).rearrange("p (c b) -> p c b", b=1024)
                hi = yb[:, c:c + 1, 512:1024]
                lo = yb[:, c:c + 1, 0:512]
                nc.vector.tensor_copy(hi, s3)
                nc.vector.tensor_tensor(lo, s3, hi, mybir.AluOpType.subtract)

            # XBAR path for chunks 0-3 (latency fully hidden: issue at h8,
            # convert at h12) -- saves the PE transpose cycles there
            yt03_stage = work.tile([128, 2048], BF16, tag="yt03", bufs=1,
                                   name="yt03_stage")

            def yt03_xpose(yt_tiles):
                for qt in range(4):
                    nc.sync.dma_start_transpose(
                        yt03_stage[:].rearrange("p (c q) -> p c q", q=512)
                        [:, :, qt * 128:(qt + 1) * 128],
                        yt_tiles[qt][:, 0:512])

            def yt03_convert(yt8):
                yb = yt8[:].bitcast(F8).rearrange("p (c b) -> p c b", b=1024)
                hi = yb[:, 0:4, 512:1024]
                lo = yb[:, 0:4, 0:512]
                src = yt03_stage[:].rearrange("p (c q) -> p c q", q=512)
                nc.vector.tensor_copy(hi, src)
                nc.vector.tensor_tensor(lo, src, hi, mybir.AluOpType.subtract)

            # prologue
            xts[0] = x_load(0)
            nc.sync.dma_start(identf[:], ident_d[:, :])
            nc.vector.tensor_copy(identb[:], identf[:])
            load_weights(0, 16)    # Q then K m-slices
            load_weights(16, 24)   # wv
            xts[1] = x_load(1)
            load_weights(24, 32)   # wp
            _mark(nc, "prologue-loads")
            make_seg_tiles(0)
            qc0 = qkv_closures(0, xts[0], qkts[0], vpss[0])
            for f in qc0[0::2][:8]:   # Q tiles
                f()
            for f in qc0[1::2][:8]:   # K tiles
                f()
            for f in qc0[16:24:2]:    # V vn=0 tiles (needed from AV(0))
                f()
            # V vn=1 (needed only from AV(8)): front of round 0's worklist
            prologue_leftover = list(qc0[17:24:2])
            _mark(nc, "prologue-qkv0")

            deferred_qkv = []
            deferred_proj = []
            yt8s = [None] * NSEG
            for r in range(NSEG):
                worklist = list(prologue_leftover) + list(deferred_qkv)
                prologue_leftover = []
                deferred_qkv = []
                # deferred proj first: they read a yt8/obs generation that
                # this round's conversions/projections will overwrite
                worklist += deferred_proj
                deferred_proj = []
                if r + 1 < NSEG:
                    make_seg_tiles(r + 1)
                    qc = qkv_closures(r + 1, xts[r + 1], qkts[r + 1],
                                      vpss[r + 1])
                    if r + 1 == NSEG - 1:
                        # defer Q/K m2-m5 and V vn=1 to the last round,
                        # which otherwise starves the PE in its tail
                        worklist += qc[0:4] + qc[12:16] + qc[16:24:2]
                        deferred_qkv = qc[4:12] + qc[17:24:2]
                    else:
                        worklist += qc
                # yt8 for THIS round's segment: filled by chunked XBAR +
                # conversions as heads complete (input always landed when
                # the DVE instruction issues -- no in-order DVE blockage)
                yt8s[r] = work.tile([128, 4096], BF16, tag="yt8", bufs=2,
                                    name=f"yt8_{r}")
                if r >= 1:
                    pc = proj_closures(r - 1, yt8s[r - 1], obs[r - 1])
                    if r < NSEG - 1:
                        worklist += pc[0:4]
                        deferred_proj = pc[4:8]
                    else:
                        worklist += pc
                _mark(nc, f"r{r}-startbatch")

                qkt, vps = qkts[r], vpss[r]
                at0s = {}
                wi = 0
                NPOP = 37 if r == NSEG - 1 else 34
                ci = [0]

                def pop_work():
                    nonlocal wi
                    ci[0] += 1
                    W = len(worklist)
                    target = (W * ci[0]) // NPOP
                    while wi < min(target, W):
                        worklist[wi]()
                        wi += 1

                last = r == NSEG - 1
                yt8_3 = yt8s[r]

                def proj3_A():
                    # first contraction half (cc 0-3) of proj(3)
                    yv3, yf3 = ytT8_views(yt8_3)
                    cls = []
                    for qt in range(4):
                        for vn in range(2):
                            def f(qt=qt, vn=vn):
                                b = next_gb()
                                for j in range(2):
                                    nc.tensor.matmul(
                                        bank(b),
                                        yv3[:, 2 * j:2 * j + 2, 1:2,
                                            qt * 128:(qt + 1) * 128],
                                        wpv4[:, 2 * j:2 * j + 2, 0:1,
                                             vn * 512:(vn + 1) * 512],
                                        start=(j == 0), stop=False,
                                        perf_mode=DR)
                                for c in range(4):
                                    nc.tensor.matmul(
                                        bank(b),
                                        yf3[:, c * 1024:(c + 1) * 1024]
                                        .rearrange("p (pl q) -> p pl q", pl=2)
                                        [:, :, qt * 128:(qt + 1) * 128],
                                        wflat(wpj, c)[
                                            :, :, vn * 512:(vn + 1) * 512],
                                        start=False, stop=(c == 3),
                                        perf_mode=DR)
                                nc.vector.tensor_scalar_mul(
                                    obs[3][qt][:, vn * 512:(vn + 1) * 512],
                                    bank(b), ODESC)
                            cls.append(f)
                    return cls

                for h in range(16):
                    at0s[h] = work.tile([128, 2048], BF16, tag="at0", bufs=3,
                                        name=f"at0_{r}_{h}")
                    st_part(qkt, h, 0)
                    exp_part(r, h, at0s[h], 0)
                    pop_work()
                    st_part(qkt, h, 1)
                    exp_part(r, h, at0s[h], 1)
                    if h >= 1:
                        ostg, rz = av_head(r, h - 1, at0s[h - 1], vps)
                        scales_head(r, h - 1, ostg, rz, yts[r])
                        del at0s[h - 1]
                        if h == 8:
                            yt03_xpose(yts[r])
                        if h >= 11 and h % 2 == 1:
                            # chunk (h-3)/2 (4..6) complete: PE transpose +
                            # convert into the AV parity bank (AV(h) claims
                            # it only at h+1; AV(h-2)'s drain already done)
                            yt_chunk_pe(yts[r], yt8s[r], (h - 3) // 2,
                                        av_bank(h))
                    if h == 13:
                        yt03_convert(yt8s[r])
                        if last:
                            worklist.extend(proj3_A())
                    pop_work()
                    _mark(nc, f"r{r}-h{h}")
                if last:
                    # kt0/kt1 accumulation can run while exp1(15) computes.
                    # One bank per qt: hardware start=True arms the whole
                    # 2KB zero region, so interleaved open groups must not
                    # share a bank.  Banks 0,1 are free (exp0(15) done) and
                    # 4,5 (AV parity pair).
                    b15 = [0, 1, 2, 3]
                    for qt in range(4):
                        for kt in range(2):
                            nc.tensor.matmul(
                                bank(b15[qt])[:, 0:65],
                                at0s[15][:, kt * 512 + qt * 128:
                                         kt * 512 + (qt + 1) * 128],
                                vps[kt][:, 15 * 66: 15 * 66 + 65],
                                start=(kt == 0), stop=False)
                    for qt in range(4):
                        for kt in range(2, 4):
                            nc.tensor.matmul(
                                bank(b15[qt])[:, 0:65],
                                at0s[15][:, kt * 512 + qt * 128:
                                         kt * 512 + (qt + 1) * 128],
                                vps[kt][:, 15 * 66: 15 * 66 + 65],
                                start=False, stop=(kt == 3))
                    rz = work.tile([128, 4], F32, tag="rz", bufs=2,
                                   name=f"rz{r}_15")
                    for qt in range(4):
                        nc.vector.reciprocal(
                            rz[:, qt:qt + 1],
                            bank(b15[qt])[:, 64:65])
                    for qt in range(4):
                        nc.vector.tensor_scalar_mul(
                            yts[r][qt][:, 15 * 64:16 * 64],
                            bank(b15[qt])[:, 0:64],
                            rz[:, qt:qt + 1])
                else:
                    ostg, rz = av_head(r, 15, at0s[15], vps)
                    scales_head(r, 15, ostg, rz, yts[r])
                yt_chunk_pe(yts[r], yt8s[r], 7, av_bank(14))
                if r + 2 < NSEG:
                    # late x load: consumers pop early next round; issuing
                    # here keeps the SP DMA queue clear for the yt03 XBAR
                    xts[r + 2] = x_load(r + 2)
                ci[0] = NPOP - 1
                pop_work()

            # epilogue: proj(3) second contraction half (cc 4-7) accumulates
            # into obs with add + descale (Y^T chunks all converted in-round).
            _mark(nc, "rounds-done")
            yv3c, yf3c = ytT8_views(yt8_3)
            bi = 0
            for qt in range(4):
                for vn in range(2):
                    widths = [384, 128] if (qt == 3 and vn == 1) else [512]
                    w0 = vn * 512
                    for wn in widths:
                        b = [4, 5, 6, 7][bi % 4]
                        bi += 1
                        for j in range(2, 4):
                            nc.tensor.matmul(
                                bank(b)[:, 0:wn],
                                yv3c[:, 2 * j:2 * j + 2, 1:2,
                                     qt * 128:(qt + 1) * 128],
                                wpv4[:, 2 * j:2 * j + 2, 0:1, w0:w0 + wn],
                                start=(j == 2), stop=False, perf_mode=DR)
                        for c in range(4, 8):
                            nc.tensor.matmul(
                                bank(b)[:, 0:wn],
                                yf3c[:, c * 1024:(c + 1) * 1024]
                                .rearrange("p (pl q) -> p pl q", pl=2)
                                [:, :, qt * 128:(qt + 1) * 128],
                                wflat(wpj, c)[:, :, w0:w0 + wn],
                                start=False, stop=(c == 7), perf_mode=DR)
                        nc.vector.scalar_tensor_tensor(
                            obs[3][qt][:, w0:w0 + wn],
                            bank(b)[:, 0:wn], ODESC,
                            obs[3][qt][:, w0:w0 + wn],
                            mybir.AluOpType.mult, mybir.AluOpType.add)
                        nc.sync.dma_start(
                            out[3 * SEG + qt * 128: 3 * SEG + (qt + 1) * 128,
                                w0:w0 + wn],
                            obs[3][qt][:, w0:w0 + wn])
                        w0 += wn

    _split_multi_waits(nc)
    return nc


_NC = None


def _hilo(t, scale):
    import ml_dtypes
    FP8 = ml_dtypes.float8_e4m3
    ts = (t.astype(np.float32) * scale)
    hi = ts.astype(FP8)
    lo = (ts - hi.astype(np.float32)).astype(FP8)
    return hi, lo


def _prep_inputs(x, w_attn, w_proj):
    """Host-side: scale, hi/lo fp8 split, transpose/layout packing."""
    import ml_dtypes
    BF = ml_dtypes.bfloat16
    x = np.asarray(x, dtype=np.float32)
    wa = np.asarray(w_attn, dtype=np.float32)
    wp = np.asarray(w_proj, dtype=np.float32)

    # x^T per core: [p, seg, c, pl(lo,hi), q] fp8 -> [128, NSEG*4096] bf16
    xh, xl = _hilo(x, 2.0 ** LGX)          # [T, C]
    def xt_layout(a):
        # [T, C] -> [128p, T//SEG seg, 8c, SEG q]
        return a.reshape(T // SEG, SEG, 8, 128).transpose(3, 0, 2, 1)
    xs = np.stack([xt_layout(xl), xt_layout(xh)], axis=3)
    # [128, nseg_total, c, pl, q] -> bytes -> bf16 cols
    xs = np.ascontiguousarray(xs).reshape(128, -1).view(np.uint16).view(BF)

    # Q/K groups: [m, c, pl(hi,lo), 128j] fp8 per partition
    wqk8 = []
    for g in range(2):
        wh, wl = _hilo(wa[:, g * C:(g + 1) * C], 2.0 ** LGW)
        def wl_layout(a):
            # [K=1024, N=1024] -> [128p, 8m, 8c, 128j]
            return a.reshape(8, 128, 8, 128).transpose(1, 2, 0, 3)
        # [p, m, c, pl, j] with pl (hi, lo)
        ws = np.stack([wl_layout(wh), wl_layout(wl)], axis=3)
        ws = np.ascontiguousarray(ws)
        wqk8.append(ws.reshape(128, -1).view(np.uint16).view(BF))

    def cpl_layout(w):
        # [K=1024, N=1024] -> [c, pl(hi,lo), n] per partition
        wh, wl = _hilo(w, 2.0 ** LGW)
        def lay(a):
            return a.reshape(8, 128, C).transpose(1, 0, 2)   # [p, c, n]
        ws = np.stack([lay(wh), lay(wl)], axis=2)            # [p, c, pl, n]
        return np.ascontiguousarray(ws).reshape(128, -1).view(np.uint16).view(BF)

    wv8 = cpl_layout(wa[:, 2 * C:3 * C])
    wp8 = cpl_layout(wp)
    return xs, wqk8, wv8, wp8


def kernel(x, w_attn, w_proj, split_sections):
    global _NC
    if _NC is None:
        _NC = _build()
    xs, wqk8, wv8, wp8 = _prep_inputs(x, w_attn, w_proj)
    # xs: [128, (T//SEG)*4096] bf16; per core slice NSEG segments
    in_maps = [
        {"xT8": np.ascontiguousarray(
            xs[:, i * NSEG * 4096:(i + 1) * NSEG * 4096]),
         "wqk8_0": wqk8[0], "wqk8_1": wqk8[1], "wv8": wv8, "wp8": wp8}
        for i in range(NCORES)
    ]
    res = run_bass_kernel_spmd(_NC, in_maps, core_ids=list(range(NCORES)))
    return np.concatenate([res.results[i]["out"] for i in range(NCORES)], axis=0)


if __name__ == "__main__":
    rng = np.random.default_rng(0)
    x = rng.standard_normal((T, C), dtype=np.float32)
    wa = (rng.standard_normal((C, 3 * C), dtype=np.float32) / np.sqrt(C)).astype(np.float32)
    wpj = (rng.standard_normal((C, C), dtype=np.float32) / np.sqrt(C)).astype(np.float32)
    y = kernel(x, wa, wpj, np.arange(1, 32) * 512)
    print("out", y.shape, y.dtype, np.abs(y).mean())


# revision 81
# speedup vs baseline: 1.0089x; 1.0032x over previous
"""MHSA over 32 independent 512-token segments, segment-parallel across 8
NeuronCores (4 segments / 2048 tokens per core, zero cross-core traffic).

QKV and output projection run as fp8e4m3 DoubleRow matmuls (0.5 cyc/row in
the PE) with hi+lo error compensation: every operand X is split host-side
into X = Xh + Xl (two fp8 planes, planar layout so the ldweights k-tile
stride meets the dual-fp8 ISA rule step%16==0), and X@W is computed as
Xh@Wh + (Xl@Wh + Xh@Wl), dropping only the lo*lo term.  Operands are
pre-scaled by powers of two (x*2^4, w*2^9) to center them in e4m3 range;
the scales are folded into the exp() argument, the A@V ones-column, and a
final tensor_scalar descale -- net precision is slightly BETTER than bf16.
A K=1024 output tile costs 12 DR insts x 256 cyc = 3072 cyc vs bf16's
8 x 512 = 4096.  x additionally arrives pre-transposed from the host
(x^T hi/lo planes), removing all on-device x transposes.

S = QK^T and A@V stay bf16 (contraction is 64/65-wide there; DoubleRow
with compensation degenerates to bf16 cost).  Per segment s, per head h:
  Q^T,K^T   DR fp8: lhsT=Wqk chunk, rhs=xT8          16x [128, 512]
  V         DR fp8, natural [tok, 1024] + s-col per head (A@V rowsum)
  S^T       = K^T' Q^T per head, 4x [64c -> 128k, 512q] into PSUM b0/b1
  A^T       = exp(S^T * 2^-26 / 8) bf16
  O'        = A^T.T @ [V_h|s]  natural [128q, 65] per qt -> bank 2+qt
  yt        = O'[:, 0:64] * recip(O'[:, 64]) per-partition scale (DVE)
  ytT8      = XBAR-transpose yt -> DVE hi/lo fp8 planes
  out       = ytT8 @ Wp8 (DR fp8) * 2^-13

PSUM banks: 0-1 S^T (ping-pong around exp), 2-5 A@V per qt, 6-7 QKV/proj
accumulation tiles.  3-stage software pipeline in PE emission order as in
the bf16 baseline (floor-paced worklist between attention heads).
"""

import numpy as np

import concourse.bass as bass
import concourse.mybir as mybir
import concourse.tile as tile
from concourse.bass_utils import run_bass_kernel_spmd

F32 = mybir.dt.float32
BF16 = mybir.dt.bfloat16
F8 = mybir.dt.float8e4
EXP = mybir.ActivationFunctionType.Exp
COPY = mybir.ActivationFunctionType.Copy
DR = mybir.MatmulPerfMode.DoubleRow

PHASE_MARKS = []


def _mark(nc, label):
    insts = list(nc.all_instructions())
    last = insts[-1].name if insts else "I-0"
    PHASE_MARKS.append((label, int(last.split("-")[1])))


T, C, H, HD = 16384, 1024, 16, 64
NCORES = 8
TOK = T // NCORES          # 2048 tokens per core
SEG = 512                  # tokens per segment
NSEG = TOK // SEG          # 4 segments per core
LGX, LGW = 4, 9            # x scaled by 2^4, weights by 2^9
LGQ = LGX + LGW            # Q/K/V scale 2^13
CY = 4                     # y_stored = y * 2^CY
VCOL = float(2.0 ** (LGQ - CY))          # ones-column value
EXPSCALE = (1.0 / np.sqrt(HD)) * (2.0 ** (-2 * LGQ))
ODESC = float(2.0 ** (-(CY + LGW)))      # final out descale


def _split_multi_waits(nc):
    """Move extra sync waits onto same-engine NoOps (1-wait ISA limit)."""
    for fn in nc.m.functions:
        for bb in fn.blocks:
            out = []
            for inst in bb.instructions:
                si = inst.sync_info
                if si is not None and si.on_wait and len(si.on_wait) > 1:
                    waits = list(si.on_wait)
                    for j, w in enumerate(waits[:-1]):
                        nop = mybir.InstNoOp(name=f"{inst.name}-wsp{j}")
                        nop.engine = inst.engine
                        nop.sync_info = mybir.SyncInfo(on_wait=[w], on_update=[])
                        out.append(nop)
                    inst.sync_info = mybir.SyncInfo(
                        on_wait=[waits[-1]], on_update=list(si.on_update)
                    )
                out.append(inst)
            bb.instructions = out


def _build():
    nc = bass.Bass("TRN2", target_bir_lowering=False, debug=False)
    # x^T hi/lo fp8, host-transposed: per seg [c(8), pl(2: lo,hi), q(512)]
    # fp8 = [128, 4096] bf16-typed; 4 segs concatenated.
    xT8d = nc.dram_tensor("xT8", [128, NSEG * 4096], BF16,
                          kind="ExternalInput").ap()
    # Q/K weights per group: [m(8), c(8), pl(2: hi,lo), 128] fp8 = [128, 8192]
    wqkd = [nc.dram_tensor(f"wqk8_{g}", [128, 8192], BF16,
                           kind="ExternalInput").ap() for g in range(2)]
    # V / proj weights: [c(8), pl(2: hi,lo), 1024] fp8 = [128, 8192]
    wvd = nc.dram_tensor("wv8", [128, 8192], BF16, kind="ExternalInput").ap()
    wpd = nc.dram_tensor("wp8", [128, 8192], BF16, kind="ExternalInput").ap()
    out = nc.dram_tensor("out", [TOK, C], F32, kind="ExternalOutput").ap()

    ident_d = nc.inline_tensor(np.eye(128, dtype=np.float32), "ident_c").ap()

    with tile.TileContext(nc) as tc:
        with (
            tc.tile_pool(name="const", bufs=1) as cpool,
            tc.tile_pool(name="wres", bufs=1) as wres,
            tc.tile_pool(name="work", bufs=1) as work,
            tc.tile_pool(name="ps", bufs=1, space="PSUM") as pspool,
        ):
            ps = pspool.tile([128, 4096], F32, tag="ps", name="ps")

            def bank(b):
                return ps[0:128, 512 * b:512 * (b + 1)]

            def bank_bf(b, j):
                return ps[0:128, 512 * b + 64 * j: 512 * b + 64 * (j + 1)].bitcast(BF16)

            # ---- constants
            identf = cpool.tile([128, 128], F32, tag="identf", name="identf")
            identb = cpool.tile([128, 128], BF16, tag="identb", name="identb")
            scratch = cpool.tile([128, 128], BF16, tag="scr", name="scr")
            nc.vector.memset(scratch[:], 1.0)
            # dummy transposes: keep PE busy (pstate ramp) while DMAs land
            for _ in range(64):
                nc.tensor.matmul(bank_bf(6, 0), scratch[:], scratch[:],
                                 is_transpose=True, start=True, stop=True,
                                 skip_group_check=True)

            # ---- resident weights (fp8 hi/lo planes in bf16-typed tiles)
            wqk = [wres.tile([128, 8192], BF16, tag=f"wqk{g}", name=f"wqk{g}")
                   for g in range(2)]
            wv = wres.tile([128, 8192], BF16, tag="wv", name="wv")
            wpj = wres.tile([128, 8192], BF16, tag="wpj", name="wpj")

            # fp8 4-dim views [p, c, pl, cols]
            def v4(t, cols):
                return t[:].bitcast(F8).rearrange(
                    "p (c pl m) -> p c pl m", c=8, pl=2)

            wqkv4 = [v4(wqk[g], 1024) for g in range(2)]
            wvv4 = v4(wv, 1024)
            wpv4 = v4(wpj, 1024)

            def wflat(t, c):
                # [p, pl(2), 1024] fp8 view of chunk c (pl: hi, lo)
                return t[:].bitcast(F8)[:, c * 2048:(c + 1) * 2048].rearrange(
                    "p (pl m) -> p pl m", pl=2)

            def load_weights(j0=0, j1=32, eng=None):
                # Q/K groups as m-column slices (contiguous src -> strided
                # dst) so prologue QKV tiles stream as slices land; V/proj
                # as plain chunk rows.
                eng = eng or nc.sync
                jobs = []
                for g in range(2):
                    for m in range(8):
                        dst = wqk[g][:].rearrange(
                            "p (c pl w) -> p c pl w", c=8, pl=2)[
                            :, :, :, m * 64:(m + 1) * 64]
                        jobs.append((wqkd[g][:, m * 1024:(m + 1) * 1024], dst))
                for cc in range(8):
                    jobs.append((wvd[:, cc * 1024:(cc + 1) * 1024],
                                 wv[:, cc * 1024:(cc + 1) * 1024]))
                for cc in range(8):
                    jobs.append((wpd[:, cc * 1024:(cc + 1) * 1024],
                                 wpj[:, cc * 1024:(cc + 1) * 1024]))
                for i, (src, dst) in list(enumerate(jobs))[j0:j1]:
                    eng.dma_start(dst, src)

            gb_state = [0]

            def next_gb():
                gb_state[0] ^= 1
                return 6 + gb_state[0]

            def x_load(s):
                t = work.tile([128, 4096], BF16, tag="xT", bufs=2,
                              name=f"xT8_{s}")
                nc.sync.dma_start(t[:], xT8d[:, s * 4096:(s + 1) * 4096])
                return t

            def xviews(xt):
                xf = xt[:].bitcast(F8)
                return (xf.rearrange("p (c pl q) -> p c pl q", c=8, pl=2), xf)

            def dr_group(bnk, lhs_main, rhs_main, lhs_cross, rhs_cross,
                         wn=512, w0=0):
                # 4 main insts (hi cc-pairs) + 8 cross insts, one psum group
                for j in range(4):
                    nc.tensor.matmul(bnk, lhs_main(j), rhs_main(j),
                                     start=(j == 0), stop=False, perf_mode=DR)
                for c in range(8):
                    nc.tensor.matmul(bnk, lhs_cross(c), rhs_cross(c),
                                     start=False, stop=(c == 7), perf_mode=DR)

            def qkv_closures(s, xt, qkt, vps):
                xv, xf = xviews(xt)
                cls = []
                for m in range(8):
                    for g in range(2):
                        def f(g=g, m=m):
                            b = next_gb()
                            dr_group(
                                bank(b),
                                lambda j, g=g, m=m: wqkv4[g][
                                    :, 2 * j:2 * j + 2, 0:1,
                                    m * 128:(m + 1) * 128],
                                lambda j: xv[:, 2 * j:2 * j + 2, 1:2, :],
                                lambda c, g=g, m=m: wflat(wqk[g], c)[
                                    :, :, m * 128:(m + 1) * 128],
                                lambda c: xf[:, c * 1024:(c + 1) * 1024]
                                .rearrange("p (pl q) -> p pl q", pl=2),
                            )
                            nc.vector.tensor_copy(
                                qkt[:, (g * 8 + m) * 512:(g * 8 + m + 1) * 512],
                                bank(b))
                        cls.append(f)
                for kt in range(4):
                    for vn in range(2):
                        def f(kt=kt, vn=vn):
                            b = next_gb()
                            dr_group(
                                bank(b),
                                lambda j, kt=kt: xv[
                                    :, 2 * j:2 * j + 2, 1:2,
                                    kt * 128:(kt + 1) * 128],
                                lambda j, vn=vn: wvv4[
                                    :, 2 * j:2 * j + 2, 0:1,
                                    vn * 512:(vn + 1) * 512],
                                lambda c, kt=kt: xf[:, c * 1024:(c + 1) * 1024]
                                .rearrange("p (pl q) -> p pl q", pl=2)
                                [:, :, kt * 128:(kt + 1) * 128],
                                lambda c, vn=vn: wflat(wv, c)[
                                    :, :, vn * 512:(vn + 1) * 512],
                            )
                            nc.vector.tensor_copy(
                                vps[kt].rearrange("p (h w) -> p h w", w=66)
                                [:, vn * 8:(vn + 1) * 8, 0:64],
                                bank(b).rearrange("p (h w) -> p h w", w=64))
                        cls.append(f)
                return cls

            def ytT8_views(yt8):
                f = yt8[:].bitcast(F8)
                return (f.rearrange("p (c pl q) -> p c pl q", c=8, pl=2), f)

            def proj_closures(s, yt8, obs, split_last=False):
                yv, yf = ytT8_views(yt8)
                cls = []
                for qt in range(4):
                    for vn in range(2):
                        def f(qt=qt, vn=vn):
                            widths = ([384, 128] if (split_last and qt == 3
                                                     and vn == 1) else [512])
                            w0 = vn * 512
                            for wn in widths:
                                b = next_gb()
                                dr_group(
                                    bank(b)[:, 0:wn],
                                    lambda j, qt=qt: yv[
                                        :, 2 * j:2 * j + 2, 1:2,
                                        qt * 128:(qt + 1) * 128],
                                    lambda j, w0=w0, wn=wn: wpv4[
                                        :, 2 * j:2 * j + 2, 0:1, w0:w0 + wn],
                                    lambda c, qt=qt: yf[
                                        :, c * 1024:(c + 1) * 1024]
                                    .rearrange("p (pl q) -> p pl q", pl=2)
                                    [:, :, qt * 128:(qt + 1) * 128],
                                    lambda c, w0=w0, wn=wn: wflat(wpj, c)[
                                        :, :, w0:w0 + wn],
                                )
                                nc.vector.tensor_scalar_mul(
                                    obs[qt][:, w0:w0 + wn], bank(b)[:, 0:wn],
                                    ODESC)
                                nc.sync.dma_start(
                                    out[s * SEG + qt * 128:
                                        s * SEG + (qt + 1) * 128,
                                        w0:w0 + wn],
                                    obs[qt][:, w0:w0 + wn])
                                w0 += wn
                        cls.append(f)
                return cls

            # ---------------- attention pieces (bf16) ----------
            # S^T double-buffered over 4 banks (part0 -> 0,1; part1 -> 2,3)
            # so st_part(h+1) never WAR-waits on exp(h); A@V packs all 4 qt
            # groups into one bank (4/5 by head parity), freeing banks 2,3.
            def st_part(qkt, h, part):
                r0 = 64 * (h % 2)
                qrow = qkt[r0:r0 + 64, (h // 2) * 512:(h // 2) * 512 + 512]
                for i in range(2):
                    kt = 2 * part + i
                    nc.tensor.matmul(
                        bank(kt),
                        qkt[r0:r0 + 64,
                            (8 + h // 2) * 512 + kt * 128:
                            (8 + h // 2) * 512 + (kt + 1) * 128],
                        qrow, start=True, stop=True)

            def exp_part(s, h, at0, part):
                nc.scalar.activation(
                    at0[:, part * 1024:(part + 1) * 1024],
                    ps[0:128, part * 1024:(part + 1) * 1024], EXP,
                    scale=EXPSCALE)

            def av_bank(h):
                return 4 + (h % 2)

            def av_head(s, h, at0, vps):
                B = 512 * av_bank(h)
                for qt in range(4):
                    for kt in range(4):
                        nc.tensor.matmul(
                            ps[0:128, B + 65 * qt: B + 65 * qt + 65],
                            at0[:, kt * 512 + qt * 128: kt * 512 + (qt + 1) * 128],
                            vps[kt][:, h * 66: h * 66 + 65],
                            start=(kt == 0), stop=(kt == 3))
                ostg = work.tile([128, 260], F32, tag="ostg", bufs=2,
                                 name=f"ostg{s}_{h}")
                nc.vector.tensor_copy(ostg[:], ps[0:128, B:B + 260])
                rz = work.tile([128, 4], F32, tag="rz", bufs=2, name=f"rz{s}_{h}")
                nc.vector.reciprocal(
                    rz[:].rearrange("p (q w) -> p q w", w=1),
                    ostg[:].rearrange("p (q w) -> p q w", w=65)[:, :, 64:65])
                return ostg, rz

            def scales_head(s, h, ostg, rz, yts):
                for qt in range(4):
                    nc.vector.tensor_scalar_mul(
                        yts[qt][:, h * 64:(h + 1) * 64],
                        ostg[:, qt * 65: qt * 65 + 64],
                        rz[:, qt:qt + 1])

            # ---------------- build the pipeline ----------------
            xts = [None] * NSEG
            qkts = [None] * NSEG
            vpss = [None] * NSEG
            yts = [None] * NSEG
            obs = [None] * NSEG

            def make_seg_tiles(s):
                qkts[s] = work.tile([128, 16 * 512], BF16, tag="qkt", bufs=2,
                                    name=f"qkt{s}")
                vpss[s] = [work.tile([128, 16 * 66], BF16, tag=f"vp{kt}", bufs=2,
                                     name=f"vp{s}_{kt}") for kt in range(4)]
                yts[s] = [work.tile([128, C], BF16, tag=f"yt{qt}", bufs=2,
                                    name=f"yt{s}_{qt}") for qt in range(4)]
                obs[s] = [work.tile([128, C], F32, tag=f"ob{qt}", bufs=1,
                                    name=f"ob{s}_{qt}") for qt in range(4)]
                for kt in range(4):
                    nc.vector.memset(
                        vpss[s][kt].rearrange("p (h w) -> p h w", w=66)[:, :, 64:65],
                        VCOL)

            def yt_chunk_pe(yt_tiles, yt8, c, b):
                # PE-transpose Y^T chunk c (4 qt tiles of [128,128] bf16)
                # into bank b (the just-drained AV parity bank), then DVE
                # hi/lo fp8 conversion straight from PSUM -- conversion
                # input is always ready (no DMA latency), so it never
                # head-blocks the DVE stream.
                for qt in range(4):
                    nc.tensor.transpose(
                        bank_bf(b, qt),
                        yt_tiles[qt][:, c * 128:(c + 1) * 128], identb[:])
                src = ps[0:128, 512 * b:512 * b + 256].bitcast(BF16)
                s3 = src.rearrange("p (c q) -> p c q", c=1)
                yb = yt8[:].bitcast(F8).rearrange("p (c b) -> p c b", b=1024)
                hi = yb[:, c:c + 1, 512:1024]
                lo = yb[:, c:c + 1, 0:512]
                nc.vector.tensor_copy(hi, s3)
                nc.vector.tensor_tensor(lo, s3, hi, mybir.AluOpType.subtract)

            # XBAR path for chunks 0-3 (latency fully hidden: issue at h8,
            # convert at h12) -- saves the PE transpose cycles there
            yt03_stage = work.tile([128, 2048], BF16, tag="yt03", bufs=1,
                                   name="yt03_stage")

            def yt03_xpose(yt_tiles):
                for qt in range(4):
                    nc.sync.dma_start_transpose(
                        yt03_stage[:].rearrange("p (c q) -> p c q", q=512)
                        [:, :, qt * 128:(qt + 1) * 128],
                        yt_tiles[qt][:, 0:512])

            def yt03_convert(yt8):
                yb = yt8[:].bitcast(F8).rearrange("p (c b) -> p c b", b=1024)
                hi = yb[:, 0:4, 512:1024]
                lo = yb[:, 0:4, 0:512]
                src = yt03_stage[:].rearrange("p (c q) -> p c q", q=512)
                nc.vector.tensor_copy(hi, src)
                nc.vector.tensor_tensor(lo, src, hi, mybir.AluOpType.subtract)

            # prologue
            xts[0] = x_load(0)
            nc.sync.dma_start(identf[:], ident_d[:, :])
            nc.vector.tensor_copy(identb[:], identf[:])
            load_weights(0, 16)    # Q then K m-slices
            load_weights(16, 24)   # wv
            xts[1] = x_load(1)
            load_weights(24, 32)   # wp
            _mark(nc, "prologue-loads")
            make_seg_tiles(0)
            qc0 = qkv_closures(0, xts[0], qkts[0], vpss[0])
            for f in qc0[0::2][:8]:   # Q tiles
                f()
            for f in qc0[1::2][:8]:   # K tiles
                f()
            for f in qc0[16:24:2]:    # V vn=0 tiles (needed from AV(0))
                f()
            # V vn=1 (needed only from AV(8)): front of round 0's worklist
            prologue_leftover = list(qc0[17:24:2])
            _mark(nc, "prologue-qkv0")

            deferred_qkv = []
            deferred_proj = []
            yt8s = [None] * NSEG
            for r in range(NSEG):
                worklist = list(prologue_leftover) + list(deferred_qkv)
                prologue_leftover = []
                deferred_qkv = []
                # deferred proj first: they read a yt8/obs generation that
                # this round's conversions/projections will overwrite
                worklist += deferred_proj
                deferred_proj = []
                if r + 1 < NSEG:
                    make_seg_tiles(r + 1)
                    qc = qkv_closures(r + 1, xts[r + 1], qkts[r + 1],
                                      vpss[r + 1])
                    if r + 1 == NSEG - 1:
                        # defer Q/K m2-m5 and V vn=1 to the last round,
                        # which otherwise starves the PE in its tail
                        worklist += qc[0:4] + qc[12:16] + qc[16:24:2]
                        deferred_qkv = qc[4:12] + qc[17:24:2]
                    else:
                        worklist += qc
                # yt8 for THIS round's segment: filled by chunked XBAR +
                # conversions as heads complete (input always landed when
                # the DVE instruction issues -- no in-order DVE blockage)
                yt8s[r] = work.tile([128, 4096], BF16, tag="yt8", bufs=2,
                                    name=f"yt8_{r}")
                if r >= 1:
                    pc = proj_closures(r - 1, yt8s[r - 1], obs[r - 1])
                    if r < NSEG - 1:
                        worklist += pc[0:4]
                        deferred_proj = pc[4:8]
                    else:
                        worklist += pc
                _mark(nc, f"r{r}-startbatch")

                qkt, vps = qkts[r], vpss[r]
                at0s = {}
                wi = 0
                NPOP = 34
                ci = [0]

                def pop_work():
                    nonlocal wi
                    ci[0] += 1
                    W = len(worklist)
                    target = (W * ci[0]) // NPOP
                    while wi < min(target, W):
                        worklist[wi]()
                        wi += 1

                last = r == NSEG - 1
                yt8_3 = yt8s[r]

                def proj3_A():
                    # first contraction half (cc 0-3) of proj(3)
                    yv3, yf3 = ytT8_views(yt8_3)
                    cls = []
                    for qt in range(4):
                        for vn in range(2):
                            def f(qt=qt, vn=vn):
                                b = next_gb()
                                for j in range(2):
                                    nc.tensor.matmul(
                                        bank(b),
                                        yv3[:, 2 * j:2 * j + 2, 1:2,
                                            qt * 128:(qt + 1) * 128],
                                        wpv4[:, 2 * j:2 * j + 2, 0:1,
                                             vn * 512:(vn + 1) * 512],
                                        start=(j == 0), stop=False,
                                        perf_mode=DR)
                                for c in range(4):
                                    nc.tensor.matmul(
                                        bank(b),
                                        yf3[:, c * 1024:(c + 1) * 1024]
                                        .rearrange("p (pl q) -> p pl q", pl=2)
                                        [:, :, qt * 128:(qt + 1) * 128],
                                        wflat(wpj, c)[
                                            :, :, vn * 512:(vn + 1) * 512],
                                        start=False, stop=(c == 3),
                                        perf_mode=DR)
                                nc.vector.tensor_scalar_mul(
                                    obs[3][qt][:, vn * 512:(vn + 1) * 512],
                                    bank(b), ODESC)
                            cls.append(f)
                    return cls

                for h in range(16):
                    at0s[h] = work.tile([128, 2048], BF16, tag="at0", bufs=3,
                                        name=f"at0_{r}_{h}")
                    st_part(qkt, h, 0)
                    exp_part(r, h, at0s[h], 0)
                    pop_work()
                    st_part(qkt, h, 1)
                    exp_part(r, h, at0s[h], 1)
                    if h >= 1:
                        ostg, rz = av_head(r, h - 1, at0s[h - 1], vps)
                        scales_head(r, h - 1, ostg, rz, yts[r])
                        del at0s[h - 1]
                        if h == 8:
                            yt03_xpose(yts[r])
                        if h >= 11 and h % 2 == 1:
                            # chunk (h-3)/2 (4..6) complete: PE transpose +
                            # convert into the AV parity bank (AV(h) claims
                            # it only at h+1; AV(h-2)'s drain already done)
                            yt_chunk_pe(yts[r], yt8s[r], (h - 3) // 2,
                                        av_bank(h))
                    if h == 13:
                        yt03_convert(yt8s[r])
                        if last:
                            worklist.extend(proj3_A())
                    pop_work()
                    _mark(nc, f"r{r}-h{h}")
                if last:
                    # kt0/kt1 accumulation can run while exp1(15) computes.
                    # One bank per qt: hardware start=True arms the whole
                    # 2KB zero region, so interleaved open groups must not
                    # share a bank.  Banks 0,1 are free (exp0(15) done) and
                    # 4,5 (AV parity pair).
                    b15 = [0, 1, 2, 3]
                    for qt in range(4):
                        for kt in range(2):
                            nc.tensor.matmul(
                                bank(b15[qt])[:, 0:65],
                                at0s[15][:, kt * 512 + qt * 128:
                                         kt * 512 + (qt + 1) * 128],
                                vps[kt][:, 15 * 66: 15 * 66 + 65],
                                start=(kt == 0), stop=False)
                    for qt in range(4):
                        for kt in range(2, 4):
                            nc.tensor.matmul(
                                bank(b15[qt])[:, 0:65],
                                at0s[15][:, kt * 512 + qt * 128:
                                         kt * 512 + (qt + 1) * 128],
                                vps[kt][:, 15 * 66: 15 * 66 + 65],
                                start=False, stop=(kt == 3))
                    rz = work.tile([128, 4], F32, tag="rz", bufs=2,
                                   name=f"rz{r}_15")
                    for qt in range(4):
                        nc.vector.reciprocal(
                            rz[:, qt:qt + 1],
                            bank(b15[qt])[:, 64:65])
                    for qt in range(4):
                        nc.vector.tensor_scalar_mul(
                            yts[r][qt][:, 15 * 64:16 * 64],
                            bank(b15[qt])[:, 0:64],
                            rz[:, qt:qt + 1])
                else:
                    ostg, rz = av_head(r, 15, at0s[15], vps)
                    scales_head(r, 15, ostg, rz, yts[r])
                yt_chunk_pe(yts[r], yt8s[r], 7, av_bank(14))
                if r + 2 < NSEG:
                    # late x load: consumers pop early next round; issuing
                    # here keeps the SP DMA queue clear for the yt03 XBAR
                    xts[r + 2] = x_load(r + 2)
                ci[0] = NPOP - 1
                pop_work()

            # epilogue: proj(3) second contraction half (cc 4-7) accumulates
            # into obs with add + descale (Y^T chunks all converted in-round).
            _mark(nc, "rounds-done")
            yv3c, yf3c = ytT8_views(yt8_3)
            bi = 0
            for qt in range(4):
                for vn in range(2):
                    widths = [384, 128] if (qt == 3 and vn == 1) else [512]
                    w0 = vn * 512
                    for wn in widths:
                        b = [4, 5, 6, 7][bi % 4]
                        bi += 1
                        for j in range(2, 4):
                            nc.tensor.matmul(
                                bank(b)[:, 0:wn],
                                yv3c[:, 2 * j:2 * j + 2, 1:2,
                                     qt * 128:(qt + 1) * 128],
                                wpv4[:, 2 * j:2 * j + 2, 0:1, w0:w0 + wn],
                                start=(j == 2), stop=False, perf_mode=DR)
                        for c in range(4, 8):
                            nc.tensor.matmul(
                                bank(b)[:, 0:wn],
                                yf3c[:, c * 1024:(c + 1) * 1024]
                                .rearrange("p (pl q) -> p pl q", pl=2)
                                [:, :, qt * 128:(qt + 1) * 128],
                                wflat(wpj, c)[:, :, w0:w0 + wn],
                                start=False, stop=(c == 7), perf_mode=DR)
                        nc.vector.scalar_tensor_tensor(
                            obs[3][qt][:, w0:w0 + wn],
                            bank(b)[:, 0:wn], ODESC,
                            obs[3][qt][:, w0:w0 + wn],
                            mybir.AluOpType.mult, mybir.AluOpType.add)
                        nc.sync.dma_start(
                            out[3 * SEG + qt * 128: 3 * SEG + (qt + 1) * 128,
                                w0:w0 + wn],
                            obs[3][qt][:, w0:w0 + wn])
                        w0 += wn

    _split_multi_waits(nc)
    return nc


_NC = None


def _hilo(t, scale):
    import ml_dtypes
    FP8 = ml_dtypes.float8_e4m3
    ts = (t.astype(np.float32) * scale)
    hi = ts.astype(FP8)
    lo = (ts - hi.astype(np.float32)).astype(FP8)
    return hi, lo


def _prep_inputs(x, w_attn, w_proj):
    """Host-side: scale, hi/lo fp8 split, transpose/layout packing."""
    import ml_dtypes
    BF = ml_dtypes.bfloat16
    x = np.asarray(x, dtype=np.float32)
    wa = np.asarray(w_attn, dtype=np.float32)
    wp = np.asarray(w_proj, dtype=np.float32)

    # x^T per core: [p, seg, c, pl(lo,hi), q] fp8 -> [128, NSEG*4096] bf16
    xh, xl = _hilo(x, 2.0 ** LGX)          # [T, C]
    def xt_layout(a):
        # [T, C] -> [128p, T//SEG seg, 8c, SEG q]
        return a.reshape(T // SEG, SEG, 8, 128).transpose(3, 0, 2, 1)
    xs = np.stack([xt_layout(xl), xt_layout(xh)], axis=3)
    # [128, nseg_total, c, pl, q] -> bytes -> bf16 cols
    xs = np.ascontiguousarray(xs).reshape(128, -1).view(np.uint16).view(BF)

    # Q/K groups: [m, c, pl(hi,lo), 128j] fp8 per partition
    wqk8 = []
    for g in range(2):
        wh, wl = _hilo(wa[:, g * C:(g + 1) * C], 2.0 ** LGW)
        def wl_layout(a):
            # [K=1024, N=1024] -> [128p, 8m, 8c, 128j]
            return a.reshape(8, 128, 8, 128).transpose(1, 2, 0, 3)
        # [p, m, c, pl, j] with pl (hi, lo)
        ws = np.stack([wl_layout(wh), wl_layout(wl)], axis=3)
        ws = np.ascontiguousarray(ws)
        wqk8.append(ws.reshape(128, -1).view(np.uint16).view(BF))

    def cpl_layout(w):
        # [K=1024, N=1024] -> [c, pl(hi,lo), n] per partition
        wh, wl = _hilo(w, 2.0 ** LGW)
        def lay(a):
            return a.reshape(8, 128, C).transpose(1, 0, 2)   # [p, c, n]
        ws = np.stack([lay(wh), lay(wl)], axis=2)            # [p, c, pl, n]
        return np.ascontiguousarray(ws).reshape(128, -1).view(np.uint16).view(BF)

    wv8 = cpl_layout(wa[:, 2 * C:3 * C])
    wp8 = cpl_layout(wp)
    return xs, wqk8, wv8, wp8


def kernel(x, w_attn, w_proj, split_sections):
    global _NC
    if _NC is None:
        _NC = _build()
    xs, wqk8, wv8, wp8 = _prep_inputs(x, w_attn, w_proj)
    # xs: [128, (T//SEG)*4096] bf16; per core slice NSEG segments
    in_maps = [
        {"xT8": np.ascontiguousarray(
            xs[:, i * NSEG * 4096:(i + 1) * NSEG * 4096]),
         "wqk8_0": wqk8[0], "wqk8_1": wqk8[1], "wv8": wv8, "wp8": wp8}
        for i in range(NCORES)
    ]
    res = run_bass_kernel_spmd(_NC, in_maps, core_ids=list(range(NCORES)))
    return np.concatenate([res.results[i]["out"] for i in range(NCORES)], axis=0)


if __name__ == "__main__":
    rng = np.random.default_rng(0)
    x = rng.standard_normal((T, C), dtype=np.float32)
    wa = (rng.standard_normal((C, 3 * C), dtype=np.float32) / np.sqrt(C)).astype(np.float32)
    wpj = (rng.standard_normal((C, C), dtype=np.float32) / np.sqrt(C)).astype(np.float32)
    y = kernel(x, wa, wpj, np.arange(1, 32) * 512)
    print("out", y.shape, y.dtype, np.abs(y).mean())
